# revision 17
# baseline (speedup 1.0000x reference)
"""Transformer decoder block (self-attn + cross-attn + FFN, post-LN) on 8
Trainium2 NeuronCores.

Sharding: data parallel. 8 cores = 2 batches x 4 query-chunks of 512 tokens.
Each core projects K/V for its own 512-token chunk, and one fused AllGather
per attention (K^T and V concatenated in one internal DRAM buffer) shares
them across the 4 cores of the batch. Each core runs attention for its 512
queries over all 2048 keys, then WO/LN and the FFN for its own tokens.

v2 changes vs the first working version:
  - 2 fused AllGathers instead of 4, issued as early as possible; Q
    projections, causal-mask construction and the residual transpose fill
    the PE/DVE idle window while the collectives fly.
  - causal mask built on device from iota + a tiny per-core [128,16] delta
    input (replaces a 2MB per-core mask upload).
  - res1 (decoder residual) built on device from xTq via PE transpose + a
    broadcast row vector (replaces a 2MB f32 upload).
  - softmax 1/Z: per-pair DVE reciprocal straight out of the [1,CH] PSUM
    partition-sum (replaces the zall gather + DMA scatter machinery).

On-chip layouts (unchanged):
  - Projections produce Q^T/K^T as [feature, token]; scores contract dh on
    partitions with 2-head row packing (concurrent via PE row tiling).
  - V is [token, dh]; AV contracts keys on partitions with 2-head column
    packing (concurrent via PE column tiling / XBUS split).
  - Scores are computed transposed (S^T = [key, query]); exp runs on the
    scalar engine straight out of PSUM; causal mask is a multiplicative
    bf16 operand on the vector engine.
  - Residual + LayerNorm run in [token, feature] (bn_stats/bn_aggr), then a
    PE transpose produces the [feature, token] operand for the next block.
All matmuls bf16 with fp32 PSUM accumulation; residual/LN paths fp32.
"""

from contextlib import ExitStack

import numpy as np
import ml_dtypes

import concourse.bass as bass
import concourse.bacc as bacc
import concourse.mybir as mybir
import concourse.tile as tile
from concourse import bass_utils
from concourse.masks import make_identity

BF16 = mybir.dt.bfloat16
F32 = mybir.dt.float32
AF = mybir.ActivationFunctionType
OP = mybir.AluOpType

B, S, D, H, F = 2, 2048, 1024, 16, 4096
DH = 64
EPS = 1e-5
CH = 512          # tokens per core
DT = D // 128     # 8 feature tiles
NKT = S // 128    # 16 key tiles
NPAIR = H // 2    # 8 head pairs
NMT = CH // 128   # 4 token tiles per core
NFT = F // 128    # 32 FFN hidden tiles
DC = D * CH       # elements in one K^T (or V) chunk

_CACHED = None


def build():
    nc = bacc.Bacc("TRN2", target_bir_lowering=False, debug=False,
                   enable_asserts=False, num_devices=8)

    # ---- per-core DRAM I/O ----
    d_xTq = nc.dram_tensor("xTq", [D, CH], BF16, kind="ExternalInput")
    d_eTq = nc.dram_tensor("eTq", [D, CH], BF16, kind="ExternalInput")
    d_kdelta = nc.dram_tensor("kdelta", [128, NKT], F32, kind="ExternalInput")
    wnames = ["sa_wq", "sa_wk", "sa_wv", "sa_wo", "ca_wq", "ca_wk", "ca_wv", "ca_wo"]
    d_w = {n: nc.dram_tensor(n, [D, D], BF16, kind="ExternalInput") for n in wnames}
    d_w1 = nc.dram_tensor("f_w1", [D, F], BF16, kind="ExternalInput")
    d_w2 = nc.dram_tensor("f_w2", [F, D], BF16, kind="ExternalInput")
    d_bq_sa = nc.dram_tensor("sa_bq", [D], F32, kind="ExternalInput")
    d_bk_sa = nc.dram_tensor("sa_bk", [D], F32, kind="ExternalInput")
    d_bq_ca = nc.dram_tensor("ca_bq", [D], F32, kind="ExternalInput")
    d_bk_ca = nc.dram_tensor("ca_bk", [D], F32, kind="ExternalInput")
    d_b1 = nc.dram_tensor("f_b1", [F], F32, kind="ExternalInput")
    d_cvec = nc.dram_tensor("cvec", [D], BF16, kind="ExternalInput")
    d_r1vec = nc.dram_tensor("r1vec", [D], BF16, kind="ExternalInput")
    d_b2v = nc.dram_tensor("b2v", [D], BF16, kind="ExternalInput")
    d_gbt = {n: nc.dram_tensor(n, [D], BF16, kind="ExternalInput")
             for n in ["sa_g", "sa_bt", "ca_g", "ca_bt", "f_g", "f_bt"]}
    d_out = nc.dram_tensor("out", [CH, D], F32, kind="ExternalOutput")
    cc = {}
    for pfx in ("sa", "ca"):
        cc[f"{pfx}_kt_in"] = nc.dram_tensor(f"cc_{pfx}_kt_in", [D, CH], BF16,
                                            kind="Internal")
        cc[f"{pfx}_kt_out"] = nc.dram_tensor(f"cc_{pfx}_kt_out", [4 * D, CH],
                                             BF16, kind="Internal")
        cc[f"{pfx}_v_in"] = nc.dram_tensor(f"cc_{pfx}_v_in", [CH, D], BF16,
                                           kind="Internal")
        cc[f"{pfx}_v_out"] = nc.dram_tensor(f"cc_{pfx}_v_out", [S, D], BF16,
                                            kind="Internal")
    GROUPS = [[0, 1, 2, 3], [4, 5, 6, 7]]

    with tile.TileContext(nc) as tc, ExitStack() as ctx:
        const = ctx.enter_context(tc.tile_pool(name="const", bufs=1))
        wpool = ctx.enter_context(tc.tile_pool(name="wpool", bufs=8))
        qpool = ctx.enter_context(tc.tile_pool(name="qpool", bufs=16))
        resp = ctx.enter_context(tc.tile_pool(name="resp", bufs=12))
        scrp = ctx.enter_context(tc.tile_pool(name="scrp", bufs=2))
        ps_s = ctx.enter_context(tc.tile_pool(name="ps_s", bufs=2, space="PSUM"))
        ps_av = ctx.enter_context(tc.tile_pool(name="ps_av", bufs=2, space="PSUM"))
        ps_m = ctx.enter_context(tc.tile_pool(name="ps_m", bufs=3, space="PSUM"))

        ident = const.tile([128, 128], F32, tag="ident")
        make_identity(nc, ident)
        identb = const.tile([128, 128], BF16, tag="identb")
        nc.vector.tensor_copy(identb, ident)
        onescol = const.tile([128, 1], BF16, tag="onescol")
        nc.vector.memset(onescol, 1.0)
        onesrow = const.tile([1, 64], F32, tag="onesrow")
        nc.vector.memset(onesrow, 1.0)
        epst = const.tile([128, 1], F32, tag="epst")
        nc.vector.memset(epst, EPS)
        zerot = const.tile([128, 1], F32, tag="zerot")
        nc.vector.memset(zerot, 0.0)

        def bias_cols(dram, ntiles, name):
            t = const.tile([128, ntiles], F32, tag=name, name=name)
            src = bass.AP(tensor=dram.ap().tensor, offset=0,
                          ap=[[1, 128], [128, ntiles]])
            nc.sync.dma_start(out=t, in_=src)
            return t

        def bcast_row(dram, tag, name):
            t = const.tile([128, D], BF16, tag=tag, bufs=2, name=name)
            src = bass.AP(tensor=dram.ap().tensor, offset=0, ap=[[0, 128], [1, D]])
            nc.sync.dma_start(out=t, in_=src)
            return t

        bq_sa = bias_cols(d_bq_sa, DT, "bqsa")
        bk_sa = bias_cols(d_bk_sa, DT, "bksa")
        bq_ca = bias_cols(d_bq_ca, DT, "bqca")
        bk_ca = bias_cols(d_bk_ca, DT, "bkca")
        b1c = bias_cols(d_b1, NFT, "b1c")

        def layer_norm(src, g_t, bt_t, out):
            """[128, D] f32 LN along free dim; out may alias src."""
            stats = scrp.tile([128, 2, 6], F32, tag="lnstat", name="lnstat")
            for s in range(2):
                nc.vector.bn_stats(out=stats[:, s, :],
                                   in_=src[:, s * 512:(s + 1) * 512])
            mv = scrp.tile([128, 2], F32, tag="lnmv", name="lnmv")
            nc.vector.bn_aggr(out=mv, in_=stats)
            rstd = scrp.tile([128, 1], F32, tag="lnrstd", name="lnrstd")
            nc.scalar.activation(out=rstd, in_=mv[:, 1:2], func=AF.Sqrt,
                                 bias=epst, scale=1.0)
            nc.vector.reciprocal(out=rstd, in_=rstd)
            cent = scrp.tile([128, D], F32, tag="scr", name="cent")
            nc.vector.scalar_tensor_tensor(out=cent, in0=src, scalar=mv[:, 0:1],
                                           in1=g_t, op0=OP.subtract, op1=OP.mult)
            nc.vector.scalar_tensor_tensor(out=out, in0=cent, scalar=rstd,
                                           in1=bt_t, op0=OP.mult, op1=OP.add)

        def load_w8(wd, ncols=D):
            ws = []
            for k in range(DT):
                t = wpool.tile([128, ncols], BF16, tag="w", name=f"w_{k}")
                nc.sync.dma_start(out=t, in_=wd.ap()[k * 128:(k + 1) * 128, :])
                ws.append(t)
            return ws

        def projT(ws, src_tiles, bias_col, out_tag):
            """out^T [feature, token] tiles: lhsT=weight cols, rhs=src^T."""
            outs = []
            for m in range(DT):
                ps = ps_m.tile([128, CH], F32, tag="ps_m", name="projps")
                for k in range(DT):
                    nc.tensor.matmul(ps, ws[k][:, m * 128:(m + 1) * 128],
                                     src_tiles[k], start=(k == 0),
                                     stop=(k == DT - 1))
                o = qpool.tile([128, CH], BF16, tag="qt", name=f"{out_tag}{m}")
                nc.scalar.activation(out=o, in_=ps, func=AF.Identity,
                                     bias=bias_col[:, m:m + 1], scale=1.0)
                outs.append(o)
            return outs

        def attention(pfx, d_ktout, d_vout, QT, masks, pools):
            attp, kvp, vpp, ppool, zpool, zsm = pools
            aun = []
            vcur = None
            for hp in range(NPAIR):
                with nc.named_scope(f"{pfx}_pair{hp}"):
                    # K^T for this head pair from the AllGather buffer:
                    # [128 (2 heads x 64 dh), S], chunk c at rows 1024c+128hp
                    ktp = kvp.tile([128, 4, CH], BF16, tag="ktp", name="ktp")
                    nc.sync.dma_start(
                        out=ktp,
                        in_=bass.AP(tensor=d_ktout.ap().tensor,
                                    offset=128 * hp * CH,
                                    ap=[[CH, 128], [D * CH, 4], [1, CH]]))
                    ktp = ktp.rearrange("p a q -> p (a q)")
                    # V for pair-pair from the AllGather buffer
                    if hp % 2 == 0:
                        vt = vpp.tile([128, NKT, 256], BF16, tag="vpp", bufs=1,
                                      name="vpp")
                        nc.sync.dma_start(
                            out=vt,
                            in_=bass.AP(tensor=d_vout.ap().tensor,
                                        offset=(hp // 2) * 256,
                                        ap=[[D, 128], [128 * D, NKT], [1, 256]]))
                        vcur = vt.rearrange("p a q -> p (a q)")
                    voff = (hp % 2) * 128

                    qa = QT[hp][0:64, :]
                    qb = QT[hp][64:128, :]
                    pav = ps_av.tile([128, CH], F32, tag="ps_av", bufs=1,
                                     name="pav")
                    zacc = zpool.tile([128, 4 * CH], BF16, tag="zacc", bufs=1,
                                      name="zacc")
                    for kt2 in range(NKT // 2):
                        pt2 = ppool.tile([128, 4 * CH], BF16, tag="pt", name="pt")
                        for sub in range(2):
                            kt = 2 * kt2 + sub
                            pss = ps_s.tile([128, 2 * CH], F32, tag="ps_s",
                                            name="pss")
                            ksl = ktp[:, kt * 128:(kt + 1) * 128]
                            nc.tensor.matmul(pss[:, 0:CH], ksl[0:64, :], qa,
                                             start=True, stop=True)
                            nc.tensor.matmul(pss[:, CH:2 * CH], ksl[64:128, :],
                                             qb, start=True, stop=True)
                            nc.scalar.activation(
                                out=pt2[:, sub * 2 * CH:(sub + 1) * 2 * CH],
                                in_=pss, func=AF.Exp, bias=zerot,
                                scale=1.0 / np.sqrt(DH))
                        if masks is not None:
                            mk = masks(kt2)
                            ptv = pt2.rearrange("p (a q) -> p a q", a=4)
                            nc.vector.tensor_mul(
                                ptv[:, 0::2, :], ptv[:, 0::2, :], mk)
                            nc.vector.tensor_mul(
                                ptv[:, 1::2, :], ptv[:, 1::2, :], mk)
                        if kt2 == 0:
                            nc.gpsimd.tensor_copy(zacc, pt2)
                        else:
                            nc.gpsimd.tensor_add(zacc, zacc, pt2)
                        for sub in range(2):
                            kt = 2 * kt2 + sub
                            po = sub * 2 * CH
                            vsl = vcur[:, kt * 256 + voff: kt * 256 + voff + 128]
                            nc.tensor.matmul(pav[0:64, :], vsl[:, 0:64],
                                             pt2[:, po:po + CH],
                                             start=(kt == 0),
                                             stop=(kt == NKT - 1))
                            nc.tensor.matmul(pav[64:128, :], vsl[:, 64:128],
                                             pt2[:, po + CH:po + 2 * CH],
                                             start=(kt == 0),
                                             stop=(kt == NKT - 1))
                    # Z rows: partition-sum of zacc via M=1 matmuls; copy the
                    # two [1,CH] Z rows to SBUF, broadcast Z across the 64 dh
                    # partitions per head with K=1 matmuls, then one 128-lane
                    # reciprocal and a fused normalize-multiply out of PSUM.
                    # zacc layout: [ktA-hA | ktA-hB | ktB-hA | ktB-hB] x 512
                    zs = zsm.tile([1, 2 * CH], F32, tag="zs", bufs=4, name="zs")
                    for h2 in range(2):
                        zf = ps_m.tile([1, CH], F32, tag="ps_m", name="zf")
                        nc.tensor.matmul(zf, onescol,
                                         zacc[:, h2 * CH:(h2 + 1) * CH],
                                         start=True, stop=False)
                        nc.tensor.matmul(zf, onescol,
                                         zacc[:, 2 * CH + h2 * CH:
                                              2 * CH + (h2 + 1) * CH],
                                         start=False, stop=True)
                        nc.vector.tensor_copy(zs[:, h2 * CH:(h2 + 1) * CH], zf)
                    przU = ps_m.tile([128, CH], F32, tag="ps_m", name="przU")
                    nc.tensor.matmul(przU[0:64, :], onesrow[0:1, :],
                                     zs[:, 0:CH],
                                     start=True, stop=True, tile_position=(0, 0))
                    nc.tensor.matmul(przU[64:128, :], onesrow[0:1, :],
                                     zs[:, CH:2 * CH],
                                     start=True, stop=True, tile_position=(0, 64))
                    rec = zsm.tile([128, CH], F32, tag="rec", bufs=2, name="rec")
                    nc.vector.reciprocal(out=rec, in_=przU)
                    at = attp.tile([128, CH], BF16, tag="aun", name=f"aun{hp}")
                    nc.vector.tensor_mul(at, pav, rec)
                    aun.append(at)
            return aun

        def kv_local_and_ag(pfx, d_wk, d_wv, bk_col, src_tiles):
            """Project this chunk's K^T/V; AllGather each as soon as staged."""
            with nc.named_scope(f"{pfx}_kvlocal"):
                wk = load_w8(d_wk)
                for m in range(DT):
                    ps = ps_m.tile([128, CH], F32, tag="ps_m", name="lkps")
                    for k in range(DT):
                        nc.tensor.matmul(ps, wk[k][:, m * 128:(m + 1) * 128],
                                         src_tiles[k], start=(k == 0),
                                         stop=(k == DT - 1))
                    st = scrp.tile([128, CH], BF16, tag="stage", bufs=4,
                                   name="ktst")
                    nc.scalar.activation(out=st, in_=ps, func=AF.Identity,
                                         bias=bk_col[:, m:m + 1], scale=1.0)
                    nc.sync.dma_start(
                        out=cc[f"{pfx}_kt_in"].ap()[m * 128:(m + 1) * 128, :],
                        in_=st)
                nc.gpsimd.collective_compute(
                    "AllGather", mybir.AluOpType.bypass,
                    ins=[cc[f"{pfx}_kt_in"].ap()],
                    outs=[cc[f"{pfx}_kt_out"].ap()],
                    replica_groups=GROUPS)
                wv = load_w8(d_wv)
                for tt in range(NMT):
                    for n in range(2):
                        ps = ps_m.tile([128, CH], F32, tag="ps_m", name="lvps")
                        for k in range(DT):
                            nc.tensor.matmul(
                                ps, src_tiles[k][:, tt * 128:(tt + 1) * 128],
                                wv[k][:, n * 512:(n + 1) * 512],
                                start=(k == 0), stop=(k == DT - 1))
                        st = scrp.tile([128, CH], BF16, tag="stage", bufs=4,
                                       name="vst")
                        nc.scalar.activation(out=st, in_=ps, func=AF.Copy)
                        nc.sync.dma_start(
                            out=cc[f"{pfx}_v_in"].ap()[tt * 128:(tt + 1) * 128,
                                                       n * 512:(n + 1) * 512],
                            in_=st)
                nc.gpsimd.collective_compute(
                    "AllGather", mybir.AluOpType.bypass,
                    ins=[cc[f"{pfx}_v_in"].ap()],
                    outs=[cc[f"{pfx}_v_out"].ap()],
                    replica_groups=GROUPS)

        def wo_resid_ln(attnT, d_wo, resid_fn, extra_vec, g_t, bt_t, tag):
            """WO matmul + residual + LN in [token, feature]; in-place LN."""
            wo = load_w8(d_wo)
            outs = []
            for mt in range(NMT):
                pre = resp.tile([128, D], F32, tag="persist", name=f"{tag}{mt}")
                rt = resid_fn(mt)
                for n in range(2):
                    ps = ps_m.tile([128, 512], F32, tag="ps_m", name="wops")
                    for k in range(DT):
                        nc.tensor.matmul(
                            ps, attnT[k][:, mt * 128:(mt + 1) * 128],
                            wo[k][:, n * 512:(n + 1) * 512],
                            start=(k == 0), stop=(k == DT - 1))
                    nc.vector.tensor_add(pre[:, n * 512:(n + 1) * 512], ps,
                                         rt[:, n * 512:(n + 1) * 512])
                if extra_vec is not None:
                    nc.vector.tensor_add(pre, pre, extra_vec)
                layer_norm(pre, g_t, bt_t, pre)
                outs.append(pre)
            return outs

        def transposeT(x_tiles, out_tag):
            """4 [128, D] f32 token-major -> 8 [128, CH] bf16 feature-major."""
            outs = [qpool.tile([128, CH], BF16, tag="qt",
                               name=f"{out_tag}{i}") for i in range(DT)]
            for mt in range(NMT):
                for ft in range(DT):
                    pst = ps_m.tile([128, 128], F32, tag="ps_m", name="tps")
                    nc.tensor.transpose(
                        pst, x_tiles[mt][:, ft * 128:(ft + 1) * 128], ident)
                    nc.vector.tensor_copy(
                        outs[ft][:, mt * 128:(mt + 1) * 128], pst)
            return outs

        # ======== attention phases (pools released before FFN) ========
        with ExitStack() as attn_ctx:
            maskp = attn_ctx.enter_context(tc.tile_pool(name="maskp", bufs=1))
            kvp = attn_ctx.enter_context(tc.tile_pool(name="kvp", bufs=2))
            vpp = attn_ctx.enter_context(tc.tile_pool(name="vpp", bufs=1))
            ppool = attn_ctx.enter_context(tc.tile_pool(name="ppool", bufs=4))
            zpool = attn_ctx.enter_context(tc.tile_pool(name="zpool", bufs=1))
            attp = attn_ctx.enter_context(tc.tile_pool(name="attp", bufs=8))
            zsm = attn_ctx.enter_context(tc.tile_pool(name="zsm", bufs=1))
            pools = (attp, kvp, vpp, ppool, zpool, zsm)

            xq = []
            for k in range(DT):
                t = qpool.tile([128, CH], BF16, tag="qt", name=f"xq{k}")
                nc.sync.dma_start(out=t, in_=d_xTq.ap()[k * 128:(k + 1) * 128, :])
                xq.append(t)
            # local K/V + fused AllGather for both attentions, issued up
            # front so the collectives overlap with Q projection / mask
            # construction / the residual transpose
            kv_local_and_ag("sa", d_w["sa_wk"], d_w["sa_wv"], bk_sa, xq)
            eq = []
            for k in range(DT):
                t = qpool.tile([128, CH], BF16, tag="qt", name=f"eq{k}")
                nc.sync.dma_start(out=t, in_=d_eTq.ap()[k * 128:(k + 1) * 128, :])
                eq.append(t)
            kv_local_and_ag("ca", d_w["ca_wk"], d_w["ca_wv"], bk_ca, eq)

            # ---- on-device causal mask: mask[k, kt, q] = (q - k >= kdelta) ----
            # kdelta[:, kt] = 128*kt - 512*j  (j = this core's chunk index)
            kdel = const.tile([128, NKT], F32, tag="kdel", name="kdel")
            nc.sync.dma_start(out=kdel, in_=d_kdelta.ap())
            qmk = scrp.tile([128, CH], F32, tag="qmk", name="qmk")
            nc.gpsimd.iota(qmk, pattern=[[1, CH]], base=0,
                           channel_multiplier=-1,
                           allow_small_or_imprecise_dtypes=True)
            maskb = maskp.tile([128, NKT, CH], BF16, tag="mask", name="maskb")
            for kt in range(NKT):
                nc.vector.tensor_scalar(out=maskb[:, kt, :], in0=qmk,
                                        scalar1=kdel[:, kt:kt + 1],
                                        scalar2=None, op0=OP.is_ge)

            def masks(kt2):
                # [128, 2, CH] view covering key tiles 2*kt2, 2*kt2+1
                return maskb[:, 2 * kt2:2 * kt2 + 2, :]

            # ---- self attention ----
            with nc.named_scope("sa_q"):
                wq = load_w8(d_w["sa_wq"])
                QTsa = projT(wq, xq, bq_sa, "qsa")

            # res1 = dec^T + r1vec, built from xTq while the AGs fly
            with nc.named_scope("res1_build"):
                r1row = bcast_row(d_r1vec, "vec", "r1row")
                res1 = [resp.tile([128, D], F32, tag="persist",
                                  name=f"res1_{i}") for i in range(NMT)]
                for mt in range(NMT):
                    for ft in range(DT):
                        pst = ps_m.tile([128, 128], BF16, tag="ps_m",
                                        name="r1ps")
                        nc.tensor.transpose(
                            pst, xq[ft][:, mt * 128:(mt + 1) * 128], identb)
                        nc.vector.tensor_add(
                            res1[mt][:, ft * 128:(ft + 1) * 128], pst,
                            r1row[:, ft * 128:(ft + 1) * 128])

            attnT = attention("sa", cc["sa_kt_out"], cc["sa_v_out"],
                              QTsa, masks, pools)

            with nc.named_scope("sa_wo_ln"):
                g1 = bcast_row(d_gbt["sa_g"], "gt", "g1")
                bt1 = bcast_row(d_gbt["sa_bt"], "gt", "bt1")
                x1 = wo_resid_ln(attnT, d_w["sa_wo"], lambda mt: res1[mt],
                                 None, g1, bt1, "x1_")
                x1T = transposeT(x1, "x1T")

            # ---- cross attention ----
            with nc.named_scope("ca_q"):
                wqc = load_w8(d_w["ca_wq"])
                QTca = projT(wqc, x1T, bq_ca, "qca")
            attnTc = attention("ca", cc["ca_kt_out"], cc["ca_v_out"],
                               QTca, None, pools)
            with nc.named_scope("ca_wo_ln"):
                cvec_t = bcast_row(d_cvec, "vec", "cvec")
                g2 = bcast_row(d_gbt["ca_g"], "gt", "g2")
                bt2 = bcast_row(d_gbt["ca_bt"], "gt", "bt2")
                y1 = wo_resid_ln(attnTc, d_w["ca_wo"],
                                 lambda mt: x1[mt],
                                 cvec_t, g2, bt2, "y1_")
                y1T = transposeT(y1, "y1T")

        # ======== FFN ========
        with ExitStack() as ffn_ctx:
            hpool = ffn_ctx.enter_context(tc.tile_pool(name="hpool", bufs=32))
            w1pool = ffn_ctx.enter_context(tc.tile_pool(name="w1pool", bufs=8))
            with nc.named_scope("ffn1"):
                w1 = []
                for k in range(DT):
                    t = w1pool.tile([128, F], BF16, tag="w1", name=f"w1_{k}")
                    nc.sync.dma_start(out=t,
                                      in_=d_w1.ap()[k * 128:(k + 1) * 128, :])
                    w1.append(t)
                hT = []
                for m in range(NFT):
                    ps = ps_m.tile([128, CH], F32, tag="ps_m", name="f1ps")
                    for k in range(DT):
                        nc.tensor.matmul(ps, w1[k][:, m * 128:(m + 1) * 128],
                                         y1T[k], start=(k == 0),
                                         stop=(k == DT - 1))
                    h = hpool.tile([128, CH], BF16, tag="h", name=f"h{m}")
                    nc.scalar.activation(out=h, in_=ps, func=AF.Relu,
                                         bias=b1c[:, m:m + 1], scale=1.0)
                    hT.append(h)
            with nc.named_scope("ffn2"):
                b2v_t = bcast_row(d_b2v, "vec", "b2v")
                h2 = [resp.tile([128, D], F32, tag="persist", name=f"h2_{i}")
                      for i in range(NMT)]
                for n in range(2):
                    pss = [ps_s.tile([128, 2 * CH], F32, tag="ps_s",
                                     name=f"f2ps{n}_{i}") for i in range(2)]
                    for kb in range(4):
                        w2b = w1pool.tile([128, 8, 512], BF16, tag="w1",
                                          name=f"w2b{kb}")
                        nc.sync.dma_start(
                            out=w2b,
                            in_=bass.AP(tensor=d_w2.ap().tensor,
                                        offset=kb * 8 * 128 * D + n * 512,
                                        ap=[[D, 128], [128 * D, 8], [1, 512]]))
                        for ks in range(8):
                            k = kb * 8 + ks
                            for mt in range(NMT):
                                nc.tensor.matmul(
                                    pss[mt // 2][:, (mt % 2) * CH:
                                                 (mt % 2 + 1) * CH],
                                    hT[k][:, mt * 128:(mt + 1) * 128],
                                    w2b[:, ks, :],
                                    start=(k == 0), stop=(k == NFT - 1))
                    for mt in range(NMT):
                        nc.vector.tensor_add(
                            h2[mt][:, n * 512:(n + 1) * 512],
                            pss[mt // 2][:, (mt % 2) * CH:(mt % 2 + 1) * CH],
                            y1[mt][:, n * 512:(n + 1) * 512])
            with nc.named_scope("ln3_out"):
                g3 = bcast_row(d_gbt["f_g"], "gt", "g3")
                bt3 = bcast_row(d_gbt["f_bt"], "gt", "bt3")
                for mt in range(NMT):
                    nc.vector.tensor_add(h2[mt], h2[mt], b2v_t)
                    layer_norm(h2[mt], g3, bt3, h2[mt])
                    nc.sync.dma_start(out=d_out.ap()[mt * 128:(mt + 1) * 128, :],
                                      in_=h2[mt])

    nc.compile()
    return nc


def _bf(a):
    return np.ascontiguousarray(a, dtype=np.float32).astype(ml_dtypes.bfloat16)


def kernel(**inputs):
    global _CACHED
    if _CACHED is None:
        _CACHED = build()
    nc = _CACHED

    f = {k: np.asarray(v, dtype=np.float32) for k, v in inputs.items()}
    dec, enc = f["decoder_input"], f["encoder_output"]
    cvec = (f["ca_bv"] @ f["ca_wo"] + f["ca_bo"]).astype(np.float32)
    r1vec = (f["sa_bv"] @ f["sa_wo"] + f["sa_bo"]).astype(np.float32)

    shared = {n: _bf(f[n]) for n in
              ["sa_wq", "sa_wk", "sa_wv", "sa_wo",
               "ca_wq", "ca_wk", "ca_wv", "ca_wo", "f_w1", "f_w2"]}
    shared.update({n: f[n] for n in ["sa_bq", "sa_bk", "ca_bq", "ca_bk", "f_b1"]})
    shared["cvec"] = _bf(cvec)
    shared["r1vec"] = _bf(r1vec)
    shared["b2v"] = _bf(f["f_b2"])
    for n in ["sa_g", "sa_bt", "ca_g", "ca_bt", "f_g", "f_bt"]:
        shared[n] = _bf(f[n])

    kt_idx = np.arange(NKT, dtype=np.float32)[None, :]
    in_maps = []
    for c in range(8):
        b, j = c // 4, c % 4
        rows = slice(j * CH, (j + 1) * CH)
        m = {
            "xTq": _bf(dec[b, rows, :].T),
            "eTq": _bf(enc[b, rows, :].T),
            "kdelta": np.ascontiguousarray(
                np.broadcast_to(128.0 * kt_idx - 512.0 * j, (128, NKT)),
                dtype=np.float32),
        }
        m.update(shared)
        in_maps.append(m)

    global LAST_RES
    res = bass_utils.run_bass_kernel_spmd(nc, in_maps, core_ids=list(range(8)))
    LAST_RES = res
    out = np.empty((B, S, D), dtype=np.float32)
    for c in range(8):
        b, j = c // 4, c % 4
        out[b, j * CH:(j + 1) * CH, :] = res.results[c]["out"]
    return out


# revision 18
# speedup vs baseline: 1.1921x; 1.1921x over previous
"""Transformer decoder block (self-attn + cross-attn + FFN, post-LN) on 8
Trainium2 NeuronCores.

Sharding: data parallel. 8 cores = 2 batches x 4 query-chunks of 512 tokens.
Each core projects K/V for its own 512-token chunk, and one fused AllGather
per attention (K^T and V concatenated in one internal DRAM buffer) shares
them across the 4 cores of the batch. Each core runs attention for its 512
queries over all 2048 keys, then WO/LN and the FFN for its own tokens.

v2 changes vs the first working version:
  - 2 fused AllGathers instead of 4, issued as early as possible; Q
    projections, causal-mask construction and the residual transpose fill
    the PE/DVE idle window while the collectives fly.
  - causal mask built on device from iota + a tiny per-core [128,16] delta
    input (replaces a 2MB per-core mask upload).
  - res1 (decoder residual) built on device from xTq via PE transpose + a
    broadcast row vector (replaces a 2MB f32 upload).
  - softmax 1/Z: per-pair DVE reciprocal straight out of the [1,CH] PSUM
    partition-sum (replaces the zall gather + DMA scatter machinery).

On-chip layouts (unchanged):
  - Projections produce Q^T/K^T as [feature, token]; scores contract dh on
    partitions with 2-head row packing (concurrent via PE row tiling).
  - V is [token, dh]; AV contracts keys on partitions with 2-head column
    packing (concurrent via PE column tiling / XBUS split).
  - Scores are computed transposed (S^T = [key, query]); exp runs on the
    scalar engine straight out of PSUM; causal mask is a multiplicative
    bf16 operand on the vector engine.
  - Residual + LayerNorm run in [token, feature] (bn_stats/bn_aggr), then a
    PE transpose produces the [feature, token] operand for the next block.
All matmuls bf16 with fp32 PSUM accumulation; residual/LN paths fp32.
"""

from contextlib import ExitStack

import numpy as np
import ml_dtypes

import concourse.bass as bass
import concourse.bacc as bacc
import concourse.mybir as mybir
import concourse.tile as tile
from concourse import bass_utils
from concourse.masks import make_identity

BF16 = mybir.dt.bfloat16
F32 = mybir.dt.float32
AF = mybir.ActivationFunctionType
OP = mybir.AluOpType

B, S, D, H, F = 2, 2048, 1024, 16, 4096
DH = 64
EPS = 1e-5
CH = 512          # tokens per core
DT = D // 128     # 8 feature tiles
NKT = S // 128    # 16 key tiles
NPAIR = H // 2    # 8 head pairs
NMT = CH // 128   # 4 token tiles per core
NFT = F // 128    # 32 FFN hidden tiles
DC = D * CH       # elements in one K^T (or V) chunk

_CACHED = None


def build():
    nc = bacc.Bacc("TRN2", target_bir_lowering=False, debug=False,
                   enable_asserts=False, num_devices=8)

    # ---- per-core DRAM I/O ----
    d_xTq = nc.dram_tensor("xTq", [D, CH], BF16, kind="ExternalInput")
    d_eTq = nc.dram_tensor("eTq", [D, CH], BF16, kind="ExternalInput")
    d_kdelta = nc.dram_tensor("kdelta", [128, NKT], F32, kind="ExternalInput")
    wnames = ["sa_wq", "sa_wk", "sa_wv", "sa_wo", "ca_wq", "ca_wk", "ca_wv", "ca_wo"]
    d_w = {n: nc.dram_tensor(n, [D, D], BF16, kind="ExternalInput") for n in wnames}
    d_w1 = nc.dram_tensor("f_w1", [D, F], BF16, kind="ExternalInput")
    d_w2 = nc.dram_tensor("f_w2", [F, D], BF16, kind="ExternalInput")
    d_bq_sa = nc.dram_tensor("sa_bq", [D], F32, kind="ExternalInput")
    d_bk_sa = nc.dram_tensor("sa_bk", [D], F32, kind="ExternalInput")
    d_bq_ca = nc.dram_tensor("ca_bq", [D], F32, kind="ExternalInput")
    d_bk_ca = nc.dram_tensor("ca_bk", [D], F32, kind="ExternalInput")
    d_b1 = nc.dram_tensor("f_b1", [F], F32, kind="ExternalInput")
    d_cvec = nc.dram_tensor("cvec", [D], BF16, kind="ExternalInput")
    d_r1vec = nc.dram_tensor("r1vec", [D], BF16, kind="ExternalInput")
    d_b2v = nc.dram_tensor("b2v", [D], BF16, kind="ExternalInput")
    d_gbt = {n: nc.dram_tensor(n, [D], BF16, kind="ExternalInput")
             for n in ["sa_g", "sa_bt", "ca_g", "ca_bt", "f_g", "f_bt"]}
    d_out = nc.dram_tensor("out", [CH, D], F32, kind="ExternalOutput")
    cc = {}
    for pfx in ("sa", "ca"):
        cc[f"{pfx}_kt_in"] = nc.dram_tensor(f"cc_{pfx}_kt_in", [D, CH], BF16,
                                            kind="Internal")
        cc[f"{pfx}_kt_out"] = nc.dram_tensor(f"cc_{pfx}_kt_out", [4 * D, CH],
                                             BF16, kind="Internal")
        cc[f"{pfx}_v_in"] = nc.dram_tensor(f"cc_{pfx}_v_in", [CH, D], BF16,
                                           kind="Internal")
        cc[f"{pfx}_v_out"] = nc.dram_tensor(f"cc_{pfx}_v_out", [S, D], BF16,
                                            kind="Internal")
    GROUPS = [[0, 1, 2, 3], [4, 5, 6, 7]]

    with tile.TileContext(nc) as tc, ExitStack() as ctx:
        const = ctx.enter_context(tc.tile_pool(name="const", bufs=1))
        wpool = ctx.enter_context(tc.tile_pool(name="wpool", bufs=8))
        qpool = ctx.enter_context(tc.tile_pool(name="qpool", bufs=16))
        resp = ctx.enter_context(tc.tile_pool(name="resp", bufs=12))
        scrp = ctx.enter_context(tc.tile_pool(name="scrp", bufs=2))
        ps_s = ctx.enter_context(tc.tile_pool(name="ps_s", bufs=2, space="PSUM"))
        ps_av = ctx.enter_context(tc.tile_pool(name="ps_av", bufs=2, space="PSUM"))
        ps_m = ctx.enter_context(tc.tile_pool(name="ps_m", bufs=3, space="PSUM"))

        ident = const.tile([128, 128], F32, tag="ident")
        make_identity(nc, ident)
        identb = const.tile([128, 128], BF16, tag="identb")
        nc.vector.tensor_copy(identb, ident)
        onescol = const.tile([128, 1], BF16, tag="onescol")
        nc.vector.memset(onescol, 1.0)
        onesrow = const.tile([1, 64], F32, tag="onesrow")
        nc.vector.memset(onesrow, 1.0)
        epst = const.tile([128, 1], F32, tag="epst")
        nc.vector.memset(epst, EPS)
        zerot = const.tile([128, 1], F32, tag="zerot")
        nc.vector.memset(zerot, 0.0)

        def bias_cols(dram, ntiles, name):
            t = const.tile([128, ntiles], F32, tag=name, name=name)
            src = bass.AP(tensor=dram.ap().tensor, offset=0,
                          ap=[[1, 128], [128, ntiles]])
            nc.sync.dma_start(out=t, in_=src)
            return t

        def bcast_row(dram, tag, name):
            t = const.tile([128, D], BF16, tag=tag, bufs=2, name=name)
            src = bass.AP(tensor=dram.ap().tensor, offset=0, ap=[[0, 128], [1, D]])
            nc.sync.dma_start(out=t, in_=src)
            return t

        bq_sa = bias_cols(d_bq_sa, DT, "bqsa")
        bk_sa = bias_cols(d_bk_sa, DT, "bksa")
        bq_ca = bias_cols(d_bq_ca, DT, "bqca")
        bk_ca = bias_cols(d_bk_ca, DT, "bkca")
        b1c = bias_cols(d_b1, NFT, "b1c")

        def layer_norm(src, g_t, bt_t, out):
            """[128, D] f32 LN along free dim; out may alias src."""
            stats = scrp.tile([128, 2, 6], F32, tag="lnstat", name="lnstat")
            for s in range(2):
                nc.vector.bn_stats(out=stats[:, s, :],
                                   in_=src[:, s * 512:(s + 1) * 512])
            mv = scrp.tile([128, 2], F32, tag="lnmv", name="lnmv")
            nc.vector.bn_aggr(out=mv, in_=stats)
            rstd = scrp.tile([128, 1], F32, tag="lnrstd", name="lnrstd")
            nc.scalar.activation(out=rstd, in_=mv[:, 1:2], func=AF.Sqrt,
                                 bias=epst, scale=1.0)
            nc.vector.reciprocal(out=rstd, in_=rstd)
            cent = scrp.tile([128, D], F32, tag="scr", name="cent")
            nc.vector.scalar_tensor_tensor(out=cent, in0=src, scalar=mv[:, 0:1],
                                           in1=g_t, op0=OP.subtract, op1=OP.mult)
            nc.vector.scalar_tensor_tensor(out=out, in0=cent, scalar=rstd,
                                           in1=bt_t, op0=OP.mult, op1=OP.add)

        def load_w8(wd, ncols=D):
            ws = []
            for k in range(DT):
                t = wpool.tile([128, ncols], BF16, tag="w", name=f"w_{k}")
                nc.sync.dma_start(out=t, in_=wd.ap()[k * 128:(k + 1) * 128, :])
                ws.append(t)
            return ws

        def projT(ws, src_tiles, bias_col, out_tag):
            """out^T [feature, token] tiles: lhsT=weight cols, rhs=src^T."""
            outs = []
            for m in range(DT):
                ps = ps_m.tile([128, CH], F32, tag="ps_m", name="projps")
                for k in range(DT):
                    nc.tensor.matmul(ps, ws[k][:, m * 128:(m + 1) * 128],
                                     src_tiles[k], start=(k == 0),
                                     stop=(k == DT - 1))
                o = qpool.tile([128, CH], BF16, tag="qt", name=f"{out_tag}{m}")
                nc.scalar.activation(out=o, in_=ps, func=AF.Identity,
                                     bias=bias_col[:, m:m + 1], scale=1.0)
                outs.append(o)
            return outs

        def attention(pfx, d_ktout, d_vout, QT, masks, pools):
            attp, kvp, vpp, ppool, zpool, zsm = pools
            aun = []
            vcur = None
            for hp in range(NPAIR):
                with nc.named_scope(f"{pfx}_pair{hp}"):
                    # K^T for this head pair from the AllGather buffer:
                    # [128 (2 heads x 64 dh), S], chunk c at rows 1024c+128hp
                    ktp = kvp.tile([128, 4, CH], BF16, tag="ktp", name="ktp")
                    nc.sync.dma_start(
                        out=ktp,
                        in_=bass.AP(tensor=d_ktout.ap().tensor,
                                    offset=128 * hp * CH,
                                    ap=[[CH, 128], [D * CH, 4], [1, CH]]))
                    ktp = ktp.rearrange("p a q -> p (a q)")
                    # V for pair-pair from the AllGather buffer
                    if hp % 2 == 0:
                        vt = vpp.tile([128, NKT, 256], BF16, tag="vpp", bufs=1,
                                      name="vpp")
                        nc.sync.dma_start(
                            out=vt,
                            in_=bass.AP(tensor=d_vout.ap().tensor,
                                        offset=(hp // 2) * 256,
                                        ap=[[D, 128], [128 * D, NKT], [1, 256]]))
                        vcur = vt.rearrange("p a q -> p (a q)")
                    voff = (hp % 2) * 128

                    qa = QT[hp][0:64, :]
                    qb = QT[hp][64:128, :]
                    pav = ps_av.tile([128, CH], F32, tag="ps_av", bufs=1,
                                     name="pav")
                    zacc = zpool.tile([128, 4 * CH], BF16, tag="zacc", bufs=1,
                                      name="zacc")
                    for kt2 in range(NKT // 2):
                        pt2 = ppool.tile([128, 4 * CH], BF16, tag="pt", name="pt")
                        for sub in range(2):
                            kt = 2 * kt2 + sub
                            pss = ps_s.tile([128, 2 * CH], F32, tag="ps_s",
                                            name="pss")
                            ksl = ktp[:, kt * 128:(kt + 1) * 128]
                            nc.tensor.matmul(pss[:, 0:CH], ksl[0:64, :], qa,
                                             start=True, stop=True)
                            nc.tensor.matmul(pss[:, CH:2 * CH], ksl[64:128, :],
                                             qb, start=True, stop=True)
                            nc.scalar.activation(
                                out=pt2[:, sub * 2 * CH:(sub + 1) * 2 * CH],
                                in_=pss, func=AF.Exp, bias=zerot,
                                scale=1.0 / np.sqrt(DH))
                        if masks is not None:
                            mk = masks(kt2)
                            ptv = pt2.rearrange("p (a q) -> p a q", a=4)
                            nc.vector.tensor_mul(
                                ptv[:, 0::2, :], ptv[:, 0::2, :], mk)
                            nc.vector.tensor_mul(
                                ptv[:, 1::2, :], ptv[:, 1::2, :], mk)
                        if kt2 == 0:
                            nc.vector.tensor_copy(zacc, pt2)
                        else:
                            nc.vector.tensor_add(zacc, zacc, pt2)
                        for sub in range(2):
                            kt = 2 * kt2 + sub
                            po = sub * 2 * CH
                            vsl = vcur[:, kt * 256 + voff: kt * 256 + voff + 128]
                            nc.tensor.matmul(pav[0:64, :], vsl[:, 0:64],
                                             pt2[:, po:po + CH],
                                             start=(kt == 0),
                                             stop=(kt == NKT - 1))
                            nc.tensor.matmul(pav[64:128, :], vsl[:, 64:128],
                                             pt2[:, po + CH:po + 2 * CH],
                                             start=(kt == 0),
                                             stop=(kt == NKT - 1))
                    # Z rows: partition-sum of zacc via M=1 matmuls; copy the
                    # two [1,CH] Z rows to SBUF, broadcast Z across the 64 dh
                    # partitions per head with K=1 matmuls, then one 128-lane
                    # reciprocal and a fused normalize-multiply out of PSUM.
                    # zacc layout: [ktA-hA | ktA-hB | ktB-hA | ktB-hB] x 512
                    zs = zsm.tile([1, 2 * CH], F32, tag="zs", bufs=4, name="zs")
                    for h2 in range(2):
                        zf = ps_m.tile([1, CH], F32, tag="ps_m", name="zf")
                        nc.tensor.matmul(zf, onescol,
                                         zacc[:, h2 * CH:(h2 + 1) * CH],
                                         start=True, stop=False)
                        nc.tensor.matmul(zf, onescol,
                                         zacc[:, 2 * CH + h2 * CH:
                                              2 * CH + (h2 + 1) * CH],
                                         start=False, stop=True)
                        nc.vector.tensor_copy(zs[:, h2 * CH:(h2 + 1) * CH], zf)
                    przU = ps_m.tile([128, CH], F32, tag="ps_m", name="przU")
                    nc.tensor.matmul(przU[0:64, :], onesrow[0:1, :],
                                     zs[:, 0:CH],
                                     start=True, stop=True, tile_position=(0, 0))
                    nc.tensor.matmul(przU[64:128, :], onesrow[0:1, :],
                                     zs[:, CH:2 * CH],
                                     start=True, stop=True, tile_position=(0, 64))
                    rec = zsm.tile([128, CH], F32, tag="rec", bufs=2, name="rec")
                    nc.vector.reciprocal(out=rec, in_=przU)
                    at = attp.tile([128, CH], BF16, tag="aun", name=f"aun{hp}")
                    nc.vector.tensor_mul(at, pav, rec)
                    aun.append(at)
            return aun

        def kv_local_and_ag(pfx, d_wk, d_wv, bk_col, src_tiles):
            """Project this chunk's K^T/V; AllGather each as soon as staged."""
            with nc.named_scope(f"{pfx}_kvlocal"):
                wk = load_w8(d_wk)
                for m in range(DT):
                    ps = ps_m.tile([128, CH], F32, tag="ps_m", name="lkps")
                    for k in range(DT):
                        nc.tensor.matmul(ps, wk[k][:, m * 128:(m + 1) * 128],
                                         src_tiles[k], start=(k == 0),
                                         stop=(k == DT - 1))
                    st = scrp.tile([128, CH], BF16, tag="stage", bufs=4,
                                   name="ktst")
                    nc.scalar.activation(out=st, in_=ps, func=AF.Identity,
                                         bias=bk_col[:, m:m + 1], scale=1.0)
                    nc.sync.dma_start(
                        out=cc[f"{pfx}_kt_in"].ap()[m * 128:(m + 1) * 128, :],
                        in_=st)
                nc.gpsimd.collective_compute(
                    "AllGather", mybir.AluOpType.bypass,
                    ins=[cc[f"{pfx}_kt_in"].ap()],
                    outs=[cc[f"{pfx}_kt_out"].ap()],
                    replica_groups=GROUPS)
                wv = load_w8(d_wv)
                for tt in range(NMT):
                    for n in range(2):
                        ps = ps_m.tile([128, CH], F32, tag="ps_m", name="lvps")
                        for k in range(DT):
                            nc.tensor.matmul(
                                ps, src_tiles[k][:, tt * 128:(tt + 1) * 128],
                                wv[k][:, n * 512:(n + 1) * 512],
                                start=(k == 0), stop=(k == DT - 1))
                        st = scrp.tile([128, CH], BF16, tag="stage", bufs=4,
                                       name="vst")
                        nc.scalar.activation(out=st, in_=ps, func=AF.Copy)
                        nc.sync.dma_start(
                            out=cc[f"{pfx}_v_in"].ap()[tt * 128:(tt + 1) * 128,
                                                       n * 512:(n + 1) * 512],
                            in_=st)
                nc.gpsimd.collective_compute(
                    "AllGather", mybir.AluOpType.bypass,
                    ins=[cc[f"{pfx}_v_in"].ap()],
                    outs=[cc[f"{pfx}_v_out"].ap()],
                    replica_groups=GROUPS)

        def wo_resid_ln(attnT, d_wo, resid_fn, extra_vec, g_t, bt_t, tag):
            """WO matmul + residual + LN in [token, feature]; in-place LN."""
            wo = load_w8(d_wo)
            outs = []
            for mt in range(NMT):
                pre = resp.tile([128, D], F32, tag="persist", name=f"{tag}{mt}")
                rt = resid_fn(mt)
                for n in range(2):
                    ps = ps_m.tile([128, 512], F32, tag="ps_m", name="wops")
                    for k in range(DT):
                        nc.tensor.matmul(
                            ps, attnT[k][:, mt * 128:(mt + 1) * 128],
                            wo[k][:, n * 512:(n + 1) * 512],
                            start=(k == 0), stop=(k == DT - 1))
                    nc.vector.tensor_add(pre[:, n * 512:(n + 1) * 512], ps,
                                         rt[:, n * 512:(n + 1) * 512])
                if extra_vec is not None:
                    nc.vector.tensor_add(pre, pre, extra_vec)
                layer_norm(pre, g_t, bt_t, pre)
                outs.append(pre)
            return outs

        def transposeT(x_tiles, out_tag):
            """4 [128, D] f32 token-major -> 8 [128, CH] bf16 feature-major."""
            outs = [qpool.tile([128, CH], BF16, tag="qt",
                               name=f"{out_tag}{i}") for i in range(DT)]
            for mt in range(NMT):
                for ft in range(DT):
                    pst = ps_m.tile([128, 128], F32, tag="ps_m", name="tps")
                    nc.tensor.transpose(
                        pst, x_tiles[mt][:, ft * 128:(ft + 1) * 128], ident)
                    nc.vector.tensor_copy(
                        outs[ft][:, mt * 128:(mt + 1) * 128], pst)
            return outs

        # ======== attention phases (pools released before FFN) ========
        with ExitStack() as attn_ctx:
            maskp = attn_ctx.enter_context(tc.tile_pool(name="maskp", bufs=1))
            kvp = attn_ctx.enter_context(tc.tile_pool(name="kvp", bufs=2))
            vpp = attn_ctx.enter_context(tc.tile_pool(name="vpp", bufs=1))
            ppool = attn_ctx.enter_context(tc.tile_pool(name="ppool", bufs=4))
            zpool = attn_ctx.enter_context(tc.tile_pool(name="zpool", bufs=1))
            attp = attn_ctx.enter_context(tc.tile_pool(name="attp", bufs=8))
            zsm = attn_ctx.enter_context(tc.tile_pool(name="zsm", bufs=1))
            pools = (attp, kvp, vpp, ppool, zpool, zsm)

            xq = []
            for k in range(DT):
                t = qpool.tile([128, CH], BF16, tag="qt", name=f"xq{k}")
                nc.sync.dma_start(out=t, in_=d_xTq.ap()[k * 128:(k + 1) * 128, :])
                xq.append(t)
            # local K/V + fused AllGather for both attentions, issued up
            # front so the collectives overlap with Q projection / mask
            # construction / the residual transpose
            kv_local_and_ag("sa", d_w["sa_wk"], d_w["sa_wv"], bk_sa, xq)
            eq = []
            for k in range(DT):
                t = qpool.tile([128, CH], BF16, tag="qt", name=f"eq{k}")
                nc.sync.dma_start(out=t, in_=d_eTq.ap()[k * 128:(k + 1) * 128, :])
                eq.append(t)
            kv_local_and_ag("ca", d_w["ca_wk"], d_w["ca_wv"], bk_ca, eq)

            # ---- on-device causal mask: mask[k, kt, q] = (q - k >= kdelta) ----
            # kdelta[:, kt] = 128*kt - 512*j  (j = this core's chunk index)
            kdel = const.tile([128, NKT], F32, tag="kdel", name="kdel")
            nc.sync.dma_start(out=kdel, in_=d_kdelta.ap())
            qmk = scrp.tile([128, CH], F32, tag="qmk", name="qmk")
            nc.gpsimd.iota(qmk, pattern=[[1, CH]], base=0,
                           channel_multiplier=-1,
                           allow_small_or_imprecise_dtypes=True)
            maskb = maskp.tile([128, NKT, CH], BF16, tag="mask", name="maskb")
            for kt in range(NKT):
                nc.vector.tensor_scalar(out=maskb[:, kt, :], in0=qmk,
                                        scalar1=kdel[:, kt:kt + 1],
                                        scalar2=None, op0=OP.is_ge)

            def masks(kt2):
                # [128, 2, CH] view covering key tiles 2*kt2, 2*kt2+1
                return maskb[:, 2 * kt2:2 * kt2 + 2, :]

            # ---- self attention ----
            with nc.named_scope("sa_q"):
                wq = load_w8(d_w["sa_wq"])
                QTsa = projT(wq, xq, bq_sa, "qsa")

            # res1 = dec^T + r1vec, built from xTq while the AGs fly
            with nc.named_scope("res1_build"):
                r1row = bcast_row(d_r1vec, "vec", "r1row")
                res1 = [resp.tile([128, D], F32, tag="persist",
                                  name=f"res1_{i}") for i in range(NMT)]
                for mt in range(NMT):
                    for ft in range(DT):
                        pst = ps_m.tile([128, 128], BF16, tag="ps_m",
                                        name="r1ps")
                        nc.tensor.transpose(
                            pst, xq[ft][:, mt * 128:(mt + 1) * 128], identb)
                        nc.vector.tensor_add(
                            res1[mt][:, ft * 128:(ft + 1) * 128], pst,
                            r1row[:, ft * 128:(ft + 1) * 128])

            attnT = attention("sa", cc["sa_kt_out"], cc["sa_v_out"],
                              QTsa, masks, pools)

            with nc.named_scope("sa_wo_ln"):
                g1 = bcast_row(d_gbt["sa_g"], "gt", "g1")
                bt1 = bcast_row(d_gbt["sa_bt"], "gt", "bt1")
                x1 = wo_resid_ln(attnT, d_w["sa_wo"], lambda mt: res1[mt],
                                 None, g1, bt1, "x1_")
                x1T = transposeT(x1, "x1T")

            # ---- cross attention ----
            with nc.named_scope("ca_q"):
                wqc = load_w8(d_w["ca_wq"])
                QTca = projT(wqc, x1T, bq_ca, "qca")
            attnTc = attention("ca", cc["ca_kt_out"], cc["ca_v_out"],
                               QTca, None, pools)
            with nc.named_scope("ca_wo_ln"):
                cvec_t = bcast_row(d_cvec, "vec", "cvec")
                g2 = bcast_row(d_gbt["ca_g"], "gt", "g2")
                bt2 = bcast_row(d_gbt["ca_bt"], "gt", "bt2")
                y1 = wo_resid_ln(attnTc, d_w["ca_wo"],
                                 lambda mt: x1[mt],
                                 cvec_t, g2, bt2, "y1_")
                y1T = transposeT(y1, "y1T")

        # ======== FFN ========
        with ExitStack() as ffn_ctx:
            hpool = ffn_ctx.enter_context(tc.tile_pool(name="hpool", bufs=32))
            w1pool = ffn_ctx.enter_context(tc.tile_pool(name="w1pool", bufs=8))
            with nc.named_scope("ffn1"):
                w1 = []
                for k in range(DT):
                    t = w1pool.tile([128, F], BF16, tag="w1", name=f"w1_{k}")
                    nc.sync.dma_start(out=t,
                                      in_=d_w1.ap()[k * 128:(k + 1) * 128, :])
                    w1.append(t)
                hT = []
                for m in range(NFT):
                    ps = ps_m.tile([128, CH], F32, tag="ps_m", name="f1ps")
                    for k in range(DT):
                        nc.tensor.matmul(ps, w1[k][:, m * 128:(m + 1) * 128],
                                         y1T[k], start=(k == 0),
                                         stop=(k == DT - 1))
                    h = hpool.tile([128, CH], BF16, tag="h", name=f"h{m}")
                    nc.scalar.activation(out=h, in_=ps, func=AF.Relu,
                                         bias=b1c[:, m:m + 1], scale=1.0)
                    hT.append(h)
            with nc.named_scope("ffn2"):
                b2v_t = bcast_row(d_b2v, "vec", "b2v")
                h2 = [resp.tile([128, D], F32, tag="persist", name=f"h2_{i}")
                      for i in range(NMT)]
                for n in range(2):
                    pss = [ps_s.tile([128, 2 * CH], F32, tag="ps_s",
                                     name=f"f2ps{n}_{i}") for i in range(2)]
                    for kb in range(4):
                        w2b = w1pool.tile([128, 8, 512], BF16, tag="w1",
                                          name=f"w2b{kb}")
                        nc.sync.dma_start(
                            out=w2b,
                            in_=bass.AP(tensor=d_w2.ap().tensor,
                                        offset=kb * 8 * 128 * D + n * 512,
                                        ap=[[D, 128], [128 * D, 8], [1, 512]]))
                        for ks in range(8):
                            k = kb * 8 + ks
                            for mt in range(NMT):
                                nc.tensor.matmul(
                                    pss[mt // 2][:, (mt % 2) * CH:
                                                 (mt % 2 + 1) * CH],
                                    hT[k][:, mt * 128:(mt + 1) * 128],
                                    w2b[:, ks, :],
                                    start=(k == 0), stop=(k == NFT - 1))
                    for mt in range(NMT):
                        nc.vector.tensor_add(
                            h2[mt][:, n * 512:(n + 1) * 512],
                            pss[mt // 2][:, (mt % 2) * CH:(mt % 2 + 1) * CH],
                            y1[mt][:, n * 512:(n + 1) * 512])
            with nc.named_scope("ln3_out"):
                g3 = bcast_row(d_gbt["f_g"], "gt", "g3")
                bt3 = bcast_row(d_gbt["f_bt"], "gt", "bt3")
                for mt in range(NMT):
                    nc.vector.tensor_add(h2[mt], h2[mt], b2v_t)
                    layer_norm(h2[mt], g3, bt3, h2[mt])
                    nc.sync.dma_start(out=d_out.ap()[mt * 128:(mt + 1) * 128, :],
                                      in_=h2[mt])

    nc.compile()
    return nc


def _bf(a):
    return np.ascontiguousarray(a, dtype=np.float32).astype(ml_dtypes.bfloat16)


def kernel(**inputs):
    global _CACHED
    if _CACHED is None:
        _CACHED = build()
    nc = _CACHED

    f = {k: np.asarray(v, dtype=np.float32) for k, v in inputs.items()}
    dec, enc = f["decoder_input"], f["encoder_output"]
    cvec = (f["ca_bv"] @ f["ca_wo"] + f["ca_bo"]).astype(np.float32)
    r1vec = (f["sa_bv"] @ f["sa_wo"] + f["sa_bo"]).astype(np.float32)

    shared = {n: _bf(f[n]) for n in
              ["sa_wq", "sa_wk", "sa_wv", "sa_wo",
               "ca_wq", "ca_wk", "ca_wv", "ca_wo", "f_w1", "f_w2"]}
    shared.update({n: f[n] for n in ["sa_bq", "sa_bk", "ca_bq", "ca_bk", "f_b1"]})
    shared["cvec"] = _bf(cvec)
    shared["r1vec"] = _bf(r1vec)
    shared["b2v"] = _bf(f["f_b2"])
    for n in ["sa_g", "sa_bt", "ca_g", "ca_bt", "f_g", "f_bt"]:
        shared[n] = _bf(f[n])

    kt_idx = np.arange(NKT, dtype=np.float32)[None, :]
    in_maps = []
    for c in range(8):
        b, j = c // 4, c % 4
        rows = slice(j * CH, (j + 1) * CH)
        m = {
            "xTq": _bf(dec[b, rows, :].T),
            "eTq": _bf(enc[b, rows, :].T),
            "kdelta": np.ascontiguousarray(
                np.broadcast_to(128.0 * kt_idx - 512.0 * j, (128, NKT)),
                dtype=np.float32),
        }
        m.update(shared)
        in_maps.append(m)

    global LAST_RES
    res = bass_utils.run_bass_kernel_spmd(nc, in_maps, core_ids=list(range(8)))
    LAST_RES = res
    out = np.empty((B, S, D), dtype=np.float32)
    for c in range(8):
        b, j = c // 4, c % 4
        out[b, j * CH:(j + 1) * CH, :] = res.results[c]["out"]
    return out


# revision 27
# speedup vs baseline: 1.4739x; 1.2364x over previous
"""Transformer decoder block (self-attn + cross-attn + FFN, post-LN) on 8
Trainium2 NeuronCores.

Sharding: data parallel. 8 cores = 2 batches x 4 query-chunks of 512 tokens.
Each core projects K/V for its own 512-token chunk, and one fused AllGather
per attention (K^T and V concatenated in one internal DRAM buffer) shares
them across the 4 cores of the batch. Each core runs attention for its 512
queries over all 2048 keys, then WO/LN and the FFN for its own tokens.

v2 changes vs the first working version:
  - 2 fused AllGathers instead of 4, issued as early as possible; Q
    projections, causal-mask construction and the residual transpose fill
    the PE/DVE idle window while the collectives fly.
  - causal mask built on device from iota + a tiny per-core [128,16] delta
    input (replaces a 2MB per-core mask upload).
  - res1 (decoder residual) built on device from xTq via PE transpose + a
    broadcast row vector (replaces a 2MB f32 upload).
  - softmax 1/Z: per-pair DVE reciprocal straight out of the [1,CH] PSUM
    partition-sum (replaces the zall gather + DMA scatter machinery).

On-chip layouts (unchanged):
  - Projections produce Q^T/K^T as [feature, token]; scores contract dh on
    partitions with 2-head row packing (concurrent via PE row tiling).
  - V is [token, dh]; AV contracts keys on partitions with 2-head column
    packing (concurrent via PE column tiling / XBUS split).
  - Scores are computed transposed (S^T = [key, query]); exp runs on the
    scalar engine straight out of PSUM; causal mask is a multiplicative
    bf16 operand on the vector engine.
  - Residual + LayerNorm run in [token, feature] (bn_stats/bn_aggr), then a
    PE transpose produces the [feature, token] operand for the next block.
All matmuls bf16 with fp32 PSUM accumulation; residual/LN paths fp32.
"""

from contextlib import ExitStack

import numpy as np
import ml_dtypes

import concourse.bass as bass
import concourse.bacc as bacc
import concourse.mybir as mybir
import concourse.tile as tile
from concourse import bass_utils
from concourse.masks import make_identity

BF16 = mybir.dt.bfloat16
F8 = mybir.dt.float8e4
F32 = mybir.dt.float32
AF = mybir.ActivationFunctionType
OP = mybir.AluOpType

B, S, D, H, F = 2, 2048, 1024, 16, 4096
DH = 64
EPS = 1e-5
CH = 512          # tokens per core
DT = D // 128     # 8 feature tiles
NKT = S // 128    # 16 key tiles
NPAIR = H // 2    # 8 head pairs
NMT = CH // 128   # 4 token tiles per core
NFT = F // 128    # 32 FFN hidden tiles
DC = D * CH       # elements in one K^T (or V) chunk

_CACHED = None


def build():
    nc = bacc.Bacc("TRN2", target_bir_lowering=False, debug=False,
                   enable_asserts=False, num_devices=8)

    # ---- per-core DRAM I/O ----
    d_xTq = nc.dram_tensor("xTq", [D, CH], BF16, kind="ExternalInput")
    d_eTq = nc.dram_tensor("eTq", [D, CH], BF16, kind="ExternalInput")
    d_kdelta = nc.dram_tensor("kdelta", [128, NKT], F32, kind="ExternalInput")
    wnames = ["sa_wq", "sa_wk", "sa_wv", "sa_wo", "ca_wq", "ca_wk", "ca_wv", "ca_wo"]
    d_w = {n: nc.dram_tensor(n, [D, D], BF16, kind="ExternalInput") for n in wnames}
    d_w1 = nc.dram_tensor("f_w1", [D, F], BF16, kind="ExternalInput")
    d_w2 = nc.dram_tensor("f_w2", [F, D], BF16, kind="ExternalInput")
    d_bq_sa = nc.dram_tensor("sa_bq", [D], F32, kind="ExternalInput")
    d_bk_sa = nc.dram_tensor("sa_bk", [D], F32, kind="ExternalInput")
    d_bq_ca = nc.dram_tensor("ca_bq", [D], F32, kind="ExternalInput")
    d_bk_ca = nc.dram_tensor("ca_bk", [D], F32, kind="ExternalInput")
    d_b1 = nc.dram_tensor("f_b1", [F], F32, kind="ExternalInput")
    d_cvec = nc.dram_tensor("cvec", [D], BF16, kind="ExternalInput")
    d_r1vec = nc.dram_tensor("r1vec", [D], BF16, kind="ExternalInput")
    d_b2v = nc.dram_tensor("b2v", [D], BF16, kind="ExternalInput")
    d_gbt = {n: nc.dram_tensor(n, [D], BF16, kind="ExternalInput")
             for n in ["sa_g", "sa_bt", "ca_g", "ca_bt", "f_g", "f_bt"]}
    d_out = nc.dram_tensor("out", [CH, D], F32, kind="ExternalOutput")
    cc = {}
    for pfx in ("sa", "ca"):
        cc[f"{pfx}_kt_in"] = nc.dram_tensor(f"cc_{pfx}_kt_in", [D, CH], F8,
                                            kind="Internal")
        cc[f"{pfx}_kt_out"] = nc.dram_tensor(f"cc_{pfx}_kt_out", [4 * D, CH],
                                             F8, kind="Internal")
        cc[f"{pfx}_v_in"] = nc.dram_tensor(f"cc_{pfx}_v_in", [CH, D], F8,
                                           kind="Internal")
        cc[f"{pfx}_v_out"] = nc.dram_tensor(f"cc_{pfx}_v_out", [S, D], F8,
                                            kind="Internal")
    GROUPS = [[0, 1, 2, 3], [4, 5, 6, 7]]

    with tile.TileContext(nc) as tc, ExitStack() as ctx:
        const = ctx.enter_context(tc.tile_pool(name="const", bufs=1))
        wpool = ctx.enter_context(tc.tile_pool(name="wpool", bufs=12))
        qpool = ctx.enter_context(tc.tile_pool(name="qpool", bufs=16))
        resp = ctx.enter_context(tc.tile_pool(name="resp", bufs=12))
        scrp = ctx.enter_context(tc.tile_pool(name="scrp", bufs=2))
        ps_s = ctx.enter_context(tc.tile_pool(name="ps_s", bufs=2, space="PSUM"))
        ps_av = ctx.enter_context(tc.tile_pool(name="ps_av", bufs=2, space="PSUM"))
        ps_m = ctx.enter_context(tc.tile_pool(name="ps_m", bufs=3, space="PSUM"))

        ident = const.tile([128, 128], F32, tag="ident")
        make_identity(nc, ident)
        identb = const.tile([128, 128], BF16, tag="identb")
        nc.vector.tensor_copy(identb, ident)
        onescol = const.tile([128, 1], BF16, tag="onescol")
        nc.vector.memset(onescol, 1.0)
        onesrow = const.tile([1, 64], F32, tag="onesrow")
        nc.vector.memset(onesrow, 1.0)
        epst = const.tile([128, 1], F32, tag="epst")
        nc.vector.memset(epst, EPS)
        zerot = const.tile([128, 1], F32, tag="zerot")
        nc.vector.memset(zerot, 0.0)

        def bias_cols(dram, ntiles, name):
            t = const.tile([128, ntiles], F32, tag=name, name=name)
            src = bass.AP(tensor=dram.ap().tensor, offset=0,
                          ap=[[1, 128], [128, ntiles]])
            nc.sync.dma_start(out=t, in_=src)
            return t

        def bcast_row(dram, tag, name):
            t = const.tile([128, D], BF16, tag=tag, bufs=2, name=name)
            src = bass.AP(tensor=dram.ap().tensor, offset=0, ap=[[0, 128], [1, D]])
            nc.sync.dma_start(out=t, in_=src)
            return t

        bq_sa = bias_cols(d_bq_sa, DT, "bqsa")
        bk_sa = bias_cols(d_bk_sa, DT, "bksa")
        bq_ca = bias_cols(d_bq_ca, DT, "bqca")
        bk_ca = bias_cols(d_bk_ca, DT, "bkca")
        b1c = bias_cols(d_b1, NFT, "b1c")

        def layer_norm(src, g_t, bt_t, out):
            """[128, D] f32 LN along free dim; out may alias src."""
            stats = scrp.tile([128, 2, 6], F32, tag="lnstat", name="lnstat")
            for s in range(2):
                nc.vector.bn_stats(out=stats[:, s, :],
                                   in_=src[:, s * 512:(s + 1) * 512])
            mv = scrp.tile([128, 2], F32, tag="lnmv", name="lnmv")
            nc.vector.bn_aggr(out=mv, in_=stats)
            rstd = scrp.tile([128, 1], F32, tag="lnrstd", name="lnrstd")
            nc.scalar.activation(out=rstd, in_=mv[:, 1:2], func=AF.Sqrt,
                                 bias=epst, scale=1.0)
            nc.vector.reciprocal(out=rstd, in_=rstd)
            cent = scrp.tile([128, D], F32, tag="scr", name="cent")
            nc.vector.scalar_tensor_tensor(out=cent, in0=src, scalar=mv[:, 0:1],
                                           in1=g_t, op0=OP.subtract, op1=OP.mult)
            nc.vector.scalar_tensor_tensor(out=out, in0=cent, scalar=rstd,
                                           in1=bt_t, op0=OP.mult, op1=OP.add)

        def load_w8(wd, ncols=D):
            ws = []
            for k in range(DT):
                t = wpool.tile([128, ncols], BF16, tag="w", name=f"w_{k}")
                nc.sync.dma_start(out=t, in_=wd.ap()[k * 128:(k + 1) * 128, :])
                ws.append(t)
            return ws

        def projT(ws, src_tiles, bias_col, out_tag):
            """out^T [feature, token] tiles: lhsT=weight cols, rhs=src^T."""
            outs = []
            for m in range(DT):
                ps = ps_m.tile([128, CH], F32, tag="ps_m", name="projps")
                for k in range(DT):
                    nc.tensor.matmul(ps, ws[k][:, m * 128:(m + 1) * 128],
                                     src_tiles[k], start=(k == 0),
                                     stop=(k == DT - 1))
                o = qpool.tile([128, CH], BF16, tag="qt", name=f"{out_tag}{m}")
                nc.scalar.activation(out=o, in_=ps, func=AF.Identity,
                                     bias=bias_col[:, m:m + 1], scale=1.0)
                outs.append(o)
            return outs

        def attention(pfx, d_ktout, d_vout, QT, amask, pools):
            attp, kvp, vpp, ppool, zpool, zsm = pools
            aun = []
            vcur = None
            causal = amask is not None
            for hp in range(NPAIR):
                with nc.named_scope(f"{pfx}_pair{hp}"):
                    # K^T for this head pair from the fp8 AllGather buffer:
                    # [128 (2 heads x 64 dh), S], chunk c at rows 1024c+128hp
                    ktp = kvp.tile([128, 4, CH], F8, tag="ktp", name="ktp")
                    nc.sync.dma_start(
                        out=ktp,
                        in_=bass.AP(tensor=d_ktout.ap().tensor,
                                    offset=128 * hp * CH,
                                    ap=[[CH, 128], [D * CH, 4], [1, CH]]))
                    ktb = kvp.tile([128, 4 * CH], BF16, tag="ktb", name="ktb")
                    nc.vector.tensor_copy(ktb, ktp.rearrange("p a q -> p (a q)"))
                    # V for pair-pair from the fp8 AllGather buffer
                    if hp % 2 == 0:
                        vt = vpp.tile([128, NKT, 256], F8, tag="vpp", bufs=1,
                                      name="vpp")
                        nc.sync.dma_start(
                            out=vt,
                            in_=bass.AP(tensor=d_vout.ap().tensor,
                                        offset=(hp // 2) * 256,
                                        ap=[[D, 128], [128 * D, NKT], [1, 256]]))
                        vb = vpp.tile([128, NKT * 256], BF16, tag="vb", bufs=1,
                                      name="vb")
                        nc.vector.tensor_copy(
                            vb, vt.rearrange("p a q -> p (a q)"))
                        vcur = vb
                    voff = (hp % 2) * 128

                    qa = QT[hp][0:64, :]
                    qb = QT[hp][64:128, :]
                    pav = ps_av.tile([128, CH], F32, tag="ps_av", bufs=1,
                                     name="pav")
                    zacc = zpool.tile([128, 4 * CH], BF16, tag="zacc", bufs=2,
                                      name="zacc")
                    for kt2 in range(NKT // 2):
                        pt2 = ppool.tile([128, 4 * CH], BF16, tag="pt", name="pt")
                        for sub in range(2):
                            kt = 2 * kt2 + sub
                            pss = ps_s.tile([128, 2 * CH], F32, tag="ps_s",
                                            name="pss")
                            ksl = ktb[:, kt * 128:(kt + 1) * 128]
                            nc.tensor.matmul(pss[:, 0:CH], ksl[0:64, :], qa,
                                             start=True, stop=not causal)
                            nc.tensor.matmul(pss[:, CH:2 * CH], ksl[64:128, :],
                                             qb, start=True, stop=not causal)
                            if causal:
                                # additive causal mask (0 / -240) folded into
                                # the scores on the PE; exp scale 1/8 turns
                                # -240 into -30 -> exp ~ 0
                                am = amask[:, kt, :]
                                nc.tensor.matmul(pss[:, 0:CH], identb, am,
                                                 start=False, stop=True,
                                                 skip_group_check=True)
                                nc.tensor.matmul(pss[:, CH:2 * CH], identb, am,
                                                 start=False, stop=True,
                                                 skip_group_check=True)
                            nc.scalar.activation(
                                out=pt2[:, sub * 2 * CH:(sub + 1) * 2 * CH],
                                in_=pss, func=AF.Exp, bias=zerot,
                                scale=1.0 / np.sqrt(DH))
                        if kt2 == 0:
                            nc.vector.tensor_copy(zacc, pt2)
                        else:
                            nc.vector.tensor_add(zacc, zacc, pt2)
                        for sub in range(2):
                            kt = 2 * kt2 + sub
                            po = sub * 2 * CH
                            vsl = vcur[:, kt * 256 + voff: kt * 256 + voff + 128]
                            nc.tensor.matmul(pav[0:64, :], vsl[:, 0:64],
                                             pt2[:, po:po + CH],
                                             start=(kt == 0),
                                             stop=(kt == NKT - 1))
                            nc.tensor.matmul(pav[64:128, :], vsl[:, 64:128],
                                             pt2[:, po + CH:po + 2 * CH],
                                             start=(kt == 0),
                                             stop=(kt == NKT - 1))
                    # free pav early with a plain copy; Z machinery and the
                    # normalize multiply run behind the next pair's pipeline.
                    at = attp.tile([128, CH], BF16, tag="aun", name=f"aun{hp}")
                    nc.vector.tensor_copy(at, pav)
                    aun.append(at)
                    # Z rows: partition-sum of zacc via M=1 matmuls; copy the
                    # two [1,CH] Z rows to SBUF, broadcast Z across the 64 dh
                    # partitions per head with K=1 matmuls, then one 128-lane
                    # reciprocal and the normalize multiply.
                    # zacc layout: [ktA-hA | ktA-hB | ktB-hA | ktB-hB] x 512
                    zs = zsm.tile([1, 2 * CH], F32, tag="zs", bufs=2, name="zs")
                    for h2 in range(2):
                        zf = ps_m.tile([1, CH], F32, tag="ps_m", name="zf")
                        nc.tensor.matmul(zf, onescol,
                                         zacc[:, h2 * CH:(h2 + 1) * CH],
                                         start=True, stop=False)
                        nc.tensor.matmul(zf, onescol,
                                         zacc[:, 2 * CH + h2 * CH:
                                              2 * CH + (h2 + 1) * CH],
                                         start=False, stop=True)
                        nc.vector.tensor_copy(zs[:, h2 * CH:(h2 + 1) * CH], zf)
                    przU = ps_m.tile([128, CH], F32, tag="ps_m", name="przU")
                    nc.tensor.matmul(przU[0:64, :], onesrow[0:1, :],
                                     zs[:, 0:CH],
                                     start=True, stop=True, tile_position=(0, 0))
                    nc.tensor.matmul(przU[64:128, :], onesrow[0:1, :],
                                     zs[:, CH:2 * CH],
                                     start=True, stop=True, tile_position=(0, 64))
                    rec = zsm.tile([128, CH], F32, tag="rec", bufs=2, name="rec")
                    nc.vector.reciprocal(out=rec, in_=przU)
                    nc.vector.tensor_mul(at, at, rec)
            return aun

        def kv_local_and_ag(pfx, d_wk, d_wv, bk_col, src_tiles):
            """Project this chunk's K^T/V; AllGather each as soon as staged."""
            with nc.named_scope(f"{pfx}_kvlocal"):
                wk = load_w8(d_wk)
                for m in range(DT):
                    ps = ps_m.tile([128, CH], F32, tag="ps_m", name="lkps")
                    for k in range(DT):
                        nc.tensor.matmul(ps, wk[k][:, m * 128:(m + 1) * 128],
                                         src_tiles[k], start=(k == 0),
                                         stop=(k == DT - 1))
                    st = scrp.tile([128, CH], F8, tag="stage", bufs=4,
                                   name="ktst")
                    nc.scalar.activation(out=st, in_=ps, func=AF.Identity,
                                         bias=bk_col[:, m:m + 1], scale=1.0)
                    nc.sync.dma_start(
                        out=cc[f"{pfx}_kt_in"].ap()[m * 128:(m + 1) * 128, :],
                        in_=st)
                nc.gpsimd.collective_compute(
                    "AllGather", mybir.AluOpType.bypass,
                    ins=[cc[f"{pfx}_kt_in"].ap()],
                    outs=[cc[f"{pfx}_kt_out"].ap()],
                    replica_groups=GROUPS)
                wv = load_w8(d_wv)
                for tt in range(NMT):
                    for n in range(2):
                        ps = ps_m.tile([128, CH], F32, tag="ps_m", name="lvps")
                        for k in range(DT):
                            nc.tensor.matmul(
                                ps, src_tiles[k][:, tt * 128:(tt + 1) * 128],
                                wv[k][:, n * 512:(n + 1) * 512],
                                start=(k == 0), stop=(k == DT - 1))
                        st = scrp.tile([128, CH], F8, tag="stage", bufs=4,
                                       name="vst")
                        nc.scalar.activation(out=st, in_=ps, func=AF.Copy)
                        nc.sync.dma_start(
                            out=cc[f"{pfx}_v_in"].ap()[tt * 128:(tt + 1) * 128,
                                                       n * 512:(n + 1) * 512],
                            in_=st)
                nc.gpsimd.collective_compute(
                    "AllGather", mybir.AluOpType.bypass,
                    ins=[cc[f"{pfx}_v_in"].ap()],
                    outs=[cc[f"{pfx}_v_out"].ap()],
                    replica_groups=GROUPS)

        def wo_resid_ln(attnT, d_wo, resid_fn, extra_vec, g_t, bt_t, tag):
            """WO matmul + residual + LN in [token, feature]; in-place LN."""
            wo = load_w8(d_wo)
            outs = []
            for mt in range(NMT):
                pre = resp.tile([128, D], F32, tag="persist", name=f"{tag}{mt}")
                rt = resid_fn(mt)
                for n in range(2):
                    ps = ps_m.tile([128, 512], F32, tag="ps_m", name="wops")
                    for k in range(DT):
                        nc.tensor.matmul(
                            ps, attnT[k][:, mt * 128:(mt + 1) * 128],
                            wo[k][:, n * 512:(n + 1) * 512],
                            start=(k == 0), stop=(k == DT - 1))
                    nc.vector.tensor_add(pre[:, n * 512:(n + 1) * 512], ps,
                                         rt[:, n * 512:(n + 1) * 512])
                if extra_vec is not None:
                    nc.vector.tensor_add(pre, pre, extra_vec)
                layer_norm(pre, g_t, bt_t, pre)
                outs.append(pre)
            return outs

        def transposeT(x_tiles, out_tag):
            """4 [128, D] f32 token-major -> 8 [128, CH] bf16 feature-major."""
            outs = [qpool.tile([128, CH], BF16, tag="qt",
                               name=f"{out_tag}{i}") for i in range(DT)]
            for mt in range(NMT):
                for ft in range(DT):
                    pst = ps_m.tile([128, 128], F32, tag="ps_m", name="tps")
                    nc.tensor.transpose(
                        pst, x_tiles[mt][:, ft * 128:(ft + 1) * 128], ident)
                    nc.vector.tensor_copy(
                        outs[ft][:, mt * 128:(mt + 1) * 128], pst)
            return outs

        # ======== attention phases (pools released before FFN) ========
        with ExitStack() as attn_ctx:
            maskp = attn_ctx.enter_context(tc.tile_pool(name="maskp", bufs=1))
            kvp = attn_ctx.enter_context(tc.tile_pool(name="kvp", bufs=2))
            vpp = attn_ctx.enter_context(tc.tile_pool(name="vpp", bufs=1))
            ppool = attn_ctx.enter_context(tc.tile_pool(name="ppool", bufs=4))
            zpool = attn_ctx.enter_context(tc.tile_pool(name="zpool", bufs=1))
            attp = attn_ctx.enter_context(tc.tile_pool(name="attp", bufs=8))
            zsm = attn_ctx.enter_context(tc.tile_pool(name="zsm", bufs=1))
            pools = (attp, kvp, vpp, ppool, zpool, zsm)

            xq = []
            for k in range(DT):
                t = qpool.tile([128, CH], BF16, tag="qt", name=f"xq{k}")
                nc.sync.dma_start(out=t, in_=d_xTq.ap()[k * 128:(k + 1) * 128, :])
                xq.append(t)
            # SA K/V + AllGathers first (each collective issued as soon as
            # its half is staged); Q projection, additive-mask construction
            # and the residual transpose fill the collective window.
            kv_local_and_ag("sa", d_w["sa_wk"], d_w["sa_wv"], bk_sa, xq)

            # ---- on-device additive causal mask:
            # amask[k, kt, q] = -240 if (q - k < kdelta[kt]) else 0, where
            # kdelta[:, kt] = 128*kt - 512*j (j = this core's chunk index).
            # Folded into the scores in PSUM by an identity matmul; the exp
            # scale 1/8 turns -240 into -30 -> exp ~ 0.
            kdel = const.tile([128, NKT], F32, tag="kdel", name="kdel")
            nc.sync.dma_start(out=kdel, in_=d_kdelta.ap())
            qmk = scrp.tile([128, CH], F32, tag="qmk", name="qmk")
            nc.gpsimd.iota(qmk, pattern=[[1, CH]], base=0,
                           channel_multiplier=-1,
                           allow_small_or_imprecise_dtypes=True)
            amask = maskp.tile([128, NKT, CH], BF16, tag="mask", name="amask")
            for kt in range(NKT):
                nc.vector.tensor_scalar(out=amask[:, kt, :], in0=qmk,
                                        scalar1=kdel[:, kt:kt + 1],
                                        scalar2=-240.0, op0=OP.is_lt,
                                        op1=OP.mult)

            # ---- self attention ----
            with nc.named_scope("sa_q"):
                wq = load_w8(d_w["sa_wq"])
                QTsa = projT(wq, xq, bq_sa, "qsa")

            # res1 = dec^T + r1vec, built from xTq while the AGs fly
            with nc.named_scope("res1_build"):
                r1row = bcast_row(d_r1vec, "vec", "r1row")
                res1 = [resp.tile([128, D], F32, tag="persist",
                                  name=f"res1_{i}") for i in range(NMT)]
                for mt in range(NMT):
                    for ft in range(DT):
                        pst = ps_m.tile([128, 128], BF16, tag="ps_m",
                                        name="r1ps")
                        nc.tensor.transpose(
                            pst, xq[ft][:, mt * 128:(mt + 1) * 128], identb)
                        nc.vector.tensor_add(
                            res1[mt][:, ft * 128:(ft + 1) * 128], pst,
                            r1row[:, ft * 128:(ft + 1) * 128])

            eq = []
            for k in range(DT):
                t = qpool.tile([128, CH], BF16, tag="qt", name=f"eq{k}")
                nc.sync.dma_start(out=t, in_=d_eTq.ap()[k * 128:(k + 1) * 128, :])
                eq.append(t)
            kv_local_and_ag("ca", d_w["ca_wk"], d_w["ca_wv"], bk_ca, eq)

            attnT = attention("sa", cc["sa_kt_out"], cc["sa_v_out"],
                              QTsa, amask, pools)

            with nc.named_scope("sa_wo_ln"):
                g1 = bcast_row(d_gbt["sa_g"], "gt", "g1")
                bt1 = bcast_row(d_gbt["sa_bt"], "gt", "bt1")
                x1 = wo_resid_ln(attnT, d_w["sa_wo"], lambda mt: res1[mt],
                                 None, g1, bt1, "x1_")
                x1T = transposeT(x1, "x1T")

            # ---- cross attention ----
            with nc.named_scope("ca_q"):
                wqc = load_w8(d_w["ca_wq"])
                QTca = projT(wqc, x1T, bq_ca, "qca")
            attnTc = attention("ca", cc["ca_kt_out"], cc["ca_v_out"],
                               QTca, None, pools)
            with nc.named_scope("ca_wo_ln"):
                cvec_t = bcast_row(d_cvec, "vec", "cvec")
                g2 = bcast_row(d_gbt["ca_g"], "gt", "g2")
                bt2 = bcast_row(d_gbt["ca_bt"], "gt", "bt2")
                y1 = wo_resid_ln(attnTc, d_w["ca_wo"],
                                 lambda mt: x1[mt],
                                 cvec_t, g2, bt2, "y1_")
                y1T = transposeT(y1, "y1T")

        # ======== FFN ========
        with ExitStack() as ffn_ctx:
            hpool = ffn_ctx.enter_context(tc.tile_pool(name="hpool", bufs=32))
            w1pool = ffn_ctx.enter_context(tc.tile_pool(name="w1pool", bufs=8))
            with nc.named_scope("ffn1"):
                w1 = []
                for k in range(DT):
                    t = w1pool.tile([128, F], BF16, tag="w1", name=f"w1_{k}")
                    nc.sync.dma_start(out=t,
                                      in_=d_w1.ap()[k * 128:(k + 1) * 128, :])
                    w1.append(t)
                hT = []
                for m in range(NFT):
                    ps = ps_m.tile([128, CH], F32, tag="ps_m", name="f1ps")
                    for k in range(DT):
                        nc.tensor.matmul(ps, w1[k][:, m * 128:(m + 1) * 128],
                                         y1T[k], start=(k == 0),
                                         stop=(k == DT - 1))
                    h = hpool.tile([128, CH], BF16, tag="h", name=f"h{m}")
                    nc.scalar.activation(out=h, in_=ps, func=AF.Relu,
                                         bias=b1c[:, m:m + 1], scale=1.0)
                    hT.append(h)
            with nc.named_scope("ffn2"):
                b2v_t = bcast_row(d_b2v, "vec", "b2v")
                h2 = [resp.tile([128, D], F32, tag="persist", name=f"h2_{i}")
                      for i in range(NMT)]
                for n in range(2):
                    pss = [ps_s.tile([128, 2 * CH], F32, tag="ps_s",
                                     name=f"f2ps{n}_{i}") for i in range(2)]
                    for kb in range(4):
                        w2b = w1pool.tile([128, 8, 512], BF16, tag="w1",
                                          name=f"w2b{kb}")
                        nc.sync.dma_start(
                            out=w2b,
                            in_=bass.AP(tensor=d_w2.ap().tensor,
                                        offset=kb * 8 * 128 * D + n * 512,
                                        ap=[[D, 128], [128 * D, 8], [1, 512]]))
                        for ks in range(8):
                            k = kb * 8 + ks
                            for mt in range(NMT):
                                nc.tensor.matmul(
                                    pss[mt // 2][:, (mt % 2) * CH:
                                                 (mt % 2 + 1) * CH],
                                    hT[k][:, mt * 128:(mt + 1) * 128],
                                    w2b[:, ks, :],
                                    start=(k == 0), stop=(k == NFT - 1))
                    for mt in range(NMT):
                        nc.vector.tensor_add(
                            h2[mt][:, n * 512:(n + 1) * 512],
                            pss[mt // 2][:, (mt % 2) * CH:(mt % 2 + 1) * CH],
                            y1[mt][:, n * 512:(n + 1) * 512])
            with nc.named_scope("ln3_out"):
                g3 = bcast_row(d_gbt["f_g"], "gt", "g3")
                bt3 = bcast_row(d_gbt["f_bt"], "gt", "bt3")
                for mt in range(NMT):
                    nc.vector.tensor_add(h2[mt], h2[mt], b2v_t)
                    layer_norm(h2[mt], g3, bt3, h2[mt])
                    nc.sync.dma_start(out=d_out.ap()[mt * 128:(mt + 1) * 128, :],
                                      in_=h2[mt])

    nc.compile()
    return nc


def _bf(a):
    return np.ascontiguousarray(a, dtype=np.float32).astype(ml_dtypes.bfloat16)


def kernel(**inputs):
    global _CACHED
    if _CACHED is None:
        _CACHED = build()
    nc = _CACHED

    f = {k: np.asarray(v, dtype=np.float32) for k, v in inputs.items()}
    dec, enc = f["decoder_input"], f["encoder_output"]
    cvec = (f["ca_bv"] @ f["ca_wo"] + f["ca_bo"]).astype(np.float32)
    r1vec = (f["sa_bv"] @ f["sa_wo"] + f["sa_bo"]).astype(np.float32)

    shared = {n: _bf(f[n]) for n in
              ["sa_wq", "sa_wk", "sa_wv", "sa_wo",
               "ca_wq", "ca_wk", "ca_wv", "ca_wo", "f_w1", "f_w2"]}
    shared.update({n: f[n] for n in ["sa_bq", "sa_bk", "ca_bq", "ca_bk", "f_b1"]})
    shared["cvec"] = _bf(cvec)
    shared["r1vec"] = _bf(r1vec)
    shared["b2v"] = _bf(f["f_b2"])
    for n in ["sa_g", "sa_bt", "ca_g", "ca_bt", "f_g", "f_bt"]:
        shared[n] = _bf(f[n])

    kt_idx = np.arange(NKT, dtype=np.float32)[None, :]
    in_maps = []
    for c in range(8):
        b, j = c // 4, c % 4
        rows = slice(j * CH, (j + 1) * CH)
        m = {
            "xTq": _bf(dec[b, rows, :].T),
            "eTq": _bf(enc[b, rows, :].T),
            "kdelta": np.ascontiguousarray(
                np.broadcast_to(128.0 * kt_idx - 512.0 * j, (128, NKT)),
                dtype=np.float32),
        }
        m.update(shared)
        in_maps.append(m)

    global LAST_RES
    res = bass_utils.run_bass_kernel_spmd(nc, in_maps, core_ids=list(range(8)))
    LAST_RES = res
    out = np.empty((B, S, D), dtype=np.float32)
    for c in range(8):
        b, j = c // 4, c % 4
        out[b, j * CH:(j + 1) * CH, :] = res.results[c]["out"]
    return out
